# revision 59
# baseline (speedup 1.0000x reference)
"""Trainium2 Bass kernel for causal multi-head attention (eval mode).

Problem shapes (hardcoded): x [B=4, S=2048, D=1024], 16 heads, head_dim 64,
weights Wq/Wk/Wv/Wo [1024, 1024], biases [1024].

reference:
  q/k/v = split_heads(x @ W.T + b)          -> [B, H, S, 64]
  scores = q k^T / 8, causal mask, softmax
  ctx = attn @ v, merge heads               -> [B, S, 1024]
  out = ctx @ Wo.T + bo

Sharding over 8 NeuronCores: core c handles batch b = c // 2 and head-group
hg = c % 2 (8 heads = 512 channels). Each core computes a partial output
[S, D] for its batch from its 8 heads; host sums the two partials per batch
and adds bo.

Per-core kernel (matmuls bf16, accumulation fp32 in PSUM):
  QT = Wq_s @ x_b^T  (+bq)   [512, S]   transposed layout, dq on partitions
  KT likewise
  V  = x_b @ Wv_s^T  (+bv)   [S, 512]   natural layout, each head's 64 cols
                                        augmented with a ones column (65)
  attention runs per head-PAIR (heads 2p, 2p+1 share a 128-partition tile):
    per kv block: ST [128 kv, 1024] holds both heads' score blocks
    (two row-group-packed matmuls, concurrent on the PE array).
    Causal narrowing: for diagonal kv blocks (window offset w >= 0) only
    q columns [w, 512) are computed/exp'd/accumulated; the 128-wide
    staircase wedge [w, w+128) is masked multiplicatively (DVE, bf16).
    P = exp(ST/8) via ACTIVATE -> bf16 (split into two per-head activations
    when w >= 256 so the ACT engine skips the unneeded prefix),
    CT'_h [65, 512] += [V_h | 1]^T P_h  (PSUM accumulate over kv blocks;
    row 64 = softmax denominator l),
    CT_h = CT'_h[0:64] * recip(l)
  out_partial = CT^T stack @ Wo_s^T  [S, D] fp32

Scheduling: the ACT engine's exp stream is the global co-bottleneck
(~167us of exp vs ~245us of PE work), so attention pair-blocks are
emitted in anti-diagonal waves over (qb, pair): (0,0), (1,0), (0,1),
(2,0), (1,1), (0,2), ...  A(qb,p) only needs qt[p] sequence block qb and
kt[p] blocks 0..qb, so the qt/kt projection is emitted in per-(t, sb)
512-column slices, each HOSTED inside an earlier attention block's kv
loop (a few matmuls per iteration, see the HOST map) — the exp stream
starts right after the first slice (~33us) and never pauses for a
standalone projection stretch.  Tiny nkb=4 blocks host nothing: their
boundaries are already dense with DVE work (masks + normalization), and
unloading them removed ~2.5us PE stalls at the following big-block
entries.  V tiles are emitted just-in-time inside the kv loop of each
q-block's first pair; out-projection tiles drain inside late kv loops
(DRAIN_FROM map) and one per pair boundary from wave 4 on.  Input DMAs
are balanced across the three DMA-capable queues (sync/scalar/gpsimd,
~100 GB/s each) in need order: wq/wk t=0 slices and the tiny biases
first on gpsimd (the first bias-add gates the exp stream), x split
3/3/2, wv right behind x (it gates the first PV accumulations), wo
last; warm-up matmuls (no data dependency) keep the PE busy and its
p-state ramped through the ~28us input window.

The pair-end normalization is ordered to free the ctp PSUM banks
immediately (both CT' copies first, then the reciprocal chain), keeping
the next pair's PV accumulation unblocked; the gpsimd partition_broadcast
(~1us) sits off the critical path, and gpsimd runs nothing else that
would force its library to reload.  The final block skips the SBUF
staging and normalizes straight out of PSUM in 128-column chunks so the
closing out-projection pipelines against it.

Softmax skips the row-max subtraction: scores/8 are O(+-10) for these
randn-scaled inputs, exp stays well inside fp32/bf16 range.
"""

from contextlib import ExitStack, nullcontext

import numpy as np
import ml_dtypes

import concourse.bacc as bacc
import concourse.bass as bass
import concourse.mybir as mybir
import concourse.tile as tile
from concourse.bass import ts
from concourse.bass_utils import run_bass_kernel_spmd

BF16 = mybir.dt.bfloat16
F32 = mybir.dt.float32
EXP = mybir.ActivationFunctionType.Exp
IDENT = mybir.ActivationFunctionType.Identity


def build_mha_nc(S=2048, D=1024, DQ=512, HD=64):
    """Build the per-core Bass program (identical on all 8 cores)."""
    H = DQ // HD          # heads per core (8)
    KC = D // 128         # contraction chunks over D (8)
    NDQ = DQ // 128       # dq tiles (4)
    NQT = S // 512        # q tiles, 512 wide (4)
    NS = S // 128         # s tiles (16)
    VW = H * (HD + 1)     # augmented V width (520)
    NPAIR = H // 2        # head pairs (4)
    SM_SCALE = 1.0 / np.sqrt(HD)

    nc = bacc.Bacc("TRN2", target_bir_lowering=False, debug=False)

    xT = nc.dram_tensor("xT", [D, S], BF16, kind="ExternalInput").ap()
    wqT = nc.dram_tensor("wqT", [D, DQ], BF16, kind="ExternalInput").ap()
    wkT = nc.dram_tensor("wkT", [D, DQ], BF16, kind="ExternalInput").ap()
    wvT = nc.dram_tensor("wvT", [D, DQ], BF16, kind="ExternalInput").ap()
    woT = nc.dram_tensor("woT", [DQ, D], BF16, kind="ExternalInput").ap()
    bq = nc.dram_tensor("bq", [DQ, 1], F32, kind="ExternalInput").ap()
    bk = nc.dram_tensor("bk", [DQ, 1], F32, kind="ExternalInput").ap()
    bv = nc.dram_tensor("bv", [1, DQ], F32, kind="ExternalInput").ap()
    out = nc.dram_tensor("out", [S, D], F32, kind="ExternalOutput").ap()

    with tile.TileContext(nc) as tc, ExitStack() as ctx:
        persist = ctx.enter_context(tc.tile_pool(name="persist", bufs=1))
        work = ctx.enter_context(tc.tile_pool(name="work", bufs=3))
        psum = ctx.enter_context(tc.tile_pool(name="psum", bufs=2, space="PSUM"))

        # ---- persistent inputs ----
        xt = [persist.tile([128, S], BF16, name=f"xt{k}", tag=f"xt{k}") for k in range(KC)]
        # wq/wk as single wide tiles so the DMA can load per-t column slices
        # (one 3D-AP DMA per t covering all 8 k-chunks)
        wqa = persist.tile([128, KC * DQ], BF16, name="wqa", tag="wqa")
        wka = persist.tile([128, KC * DQ], BF16, name="wka", tag="wka")
        wv = [persist.tile([128, DQ], BF16, name=f"wv{k}", tag=f"wv{k}") for k in range(KC)]
        wo = [persist.tile([128, D], BF16, name=f"wo{t}", tag=f"wo{t}") for t in range(NDQ)]
        bqa = persist.tile([128, NDQ], F32, name="bqa", tag="bqa")
        bka = persist.tile([128, NDQ], F32, name="bka", tag="bka")
        bvb = persist.tile([128, DQ], F32, name="bvb", tag="bvb")

        def wslice(wa, k, t):
            # [128, 128] lhsT slice of weight chunk k, output columns t
            return wa[:, k * DQ + t * 128 : k * DQ + (t + 1) * 128]

        def w_t_dma(eng, wa, wT, t):
            # one DMA loading the t-th 128-column slice of all 8 chunks
            dst = wa.rearrange("p (k c) -> p k c", c=DQ)[:, :, ts(t, 128)]
            src = bass.AP(
                tensor=wT.tensor,
                offset=t * 128,
                ap=[[DQ, 128], [DQ * 128, KC], [1, 128]],
            )
            eng.dma_start(out=dst, in_=src)

        # Input DMAs. Only sync (SP), scalar (ACT), and gpsimd can issue
        # DMAs; per-queue bandwidth is ~100 GB/s.  Balance by need time:
        # x (4 MB) split 3/3/2, the t=0 slices of wq/wk first on gpsimd so
        # the first projection can start as soon as x lands, later t slices
        # trickle in behind, wv/biases next, wo last.
        warm_in = persist.tile([128, 512], BF16, name="warm_in", tag="warm_in")
        nc.vector.memset(warm_in, 1.0)

        w_t_dma(nc.gpsimd, wqa, wqT, 0)
        w_t_dma(nc.gpsimd, wka, wkT, 0)
        # biases right behind the small t=0 weight slices on the gpsimd
        # queue (~17us): the very first bias-add gates the whole exp
        # stream, and behind the x megabytes it would land at ~37us
        nc.gpsimd.dma_start(
            out=bqa, in_=bass.AP(tensor=bq.tensor, offset=0, ap=[[1, 128], [128, NDQ]])
        )
        nc.gpsimd.dma_start(
            out=bka, in_=bass.AP(tensor=bk.tensor, offset=0, ap=[[1, 128], [128, NDQ]])
        )
        for k in range(3):
            nc.sync.dma_start(out=xt[k], in_=xT[ts(k, 128), :])
        for k in range(3, 6):
            nc.scalar.dma_start(out=xt[k], in_=xT[ts(k, 128), :])
        nc.gpsimd.dma_start(out=xt[6], in_=xT[ts(6, 128), :])
        nc.gpsimd.dma_start(out=xt[7], in_=xT[ts(7, 128), :])
        # bv: tiny single-row DMA + on-chip partition broadcast (a
        # 256 KB broadcast-DMA here would delay the PV-gating wv chunks
        # behind it on the sync queue by ~2.4us)
        bvr = persist.tile([1, DQ], F32, name="bvr", tag="bvr")
        nc.sync.dma_start(out=bvr, in_=bv[0:1, :])
        nc.gpsimd.partition_broadcast(bvb, bvr)
        # wv split 3/3/2 right behind x so the first V tiles can be
        # projected by ~33us (they gate the first block's PV stream)
        for k in range(3):
            nc.sync.dma_start(out=wv[k], in_=wvT[ts(k, 128), :])
        for k in range(3, 6):
            nc.scalar.dma_start(out=wv[k], in_=wvT[ts(k, 128), :])
        for k in range(6, KC):
            nc.gpsimd.dma_start(out=wv[k], in_=wvT[ts(k, 128), :])
        for t in range(1, NDQ):
            w_t_dma(nc.gpsimd, wqa, wqT, t)
            w_t_dma(nc.gpsimd, wka, wkT, t)
        for t in range(NDQ):
            nc.sync.dma_start(out=wo[t], in_=woT[ts(t, 128), :])

        # multiplicative causal staircase mask: M[i, c] = 1 if c >= i + 384
        # else 0. The wedge slice cmask[:, 384:512] has M[i, j] = (j >= i);
        # it is duplicated into both halves of a [128, 256] tile so one
        # 3D-AP multiply masks both heads' wedges in a single DVE op.
        cmask = persist.tile([128, 512], BF16, name="cmask", tag="cmask")
        nc.gpsimd.memset(cmask, 1.0)
        nc.gpsimd.affine_select(
            out=cmask,
            in_=cmask,
            compare_op=mybir.AluOpType.is_ge,
            fill=0.0,
            base=-384,
            pattern=[[1, 512]],
            channel_multiplier=-1,
        )
        cmaskd = persist.tile([128, 256], BF16, name="cmaskd", tag="cmaskd")
        nc.vector.tensor_copy(cmaskd[:, 0:128], cmask[:, 384:512])
        nc.vector.tensor_copy(cmaskd[:, 128:256], cmask[:, 384:512])

        # warm-up: dummy matmuls with no DMA dependency, covering the
        # input-DMA window and ramping the PE p-state. Results never read.
        def emit_warm(n):
            for _ in range(n):
                warm = psum.tile([128, 1024], F32, name="warm", tag="st", bufs=2)
                nc.tensor.matmul(
                    warm[:, 0:512],
                    lhsT=warm_in[:, 0:128],
                    rhs=warm_in,
                    start=True,
                    stop=True,
                )

        emit_warm(44)

        # ---- persistent intermediates ----
        qt = [persist.tile([128, S], BF16, name=f"qt{t}", tag=f"qt{t}") for t in range(NDQ)]
        kt = [persist.tile([128, S], BF16, name=f"kt{t}", tag=f"kt{t}") for t in range(NDQ)]
        vt = [persist.tile([128, VW], BF16, name=f"vt{s}", tag=f"vt{s}") for s in range(NS)]
        ct = [persist.tile([128, S], BF16, name=f"ct{t}", tag=f"ct{t}") for t in range(NDQ)]

        def emit_proj_sb(t, sb, interleave=False):
            # QT/KT projection for dq tile t, one 512-wide sequence block.
            # With interleave (the very first slice, racing the input DMA):
            # Q and K accumulations are opened together with the
            # xt[7]-dependent final matmuls pushed last, and warm filler is
            # woven between the early k-steps so the PE p-state stays ramped
            # while the x chunks trickle in.
            pjq = psum.tile([128, 512], F32, name="pjq", tag="acc", bufs=2)
            pjk = psum.tile([128, 512], F32, name="pjk", tag="acc", bufs=2)
            order = (
                [("q", k) for k in range(KC - 1)]
                + [("k", k) for k in range(KC - 1)]
                + [("q", KC - 1), ("k", KC - 1)]
                if interleave
                else [("q", k) for k in range(KC)] + [("k", k) for k in range(KC)]
            )
            for which, k in order:
                pj, wa = (pjq, wqa) if which == "q" else (pjk, wka)
                nc.tensor.matmul(
                    pj,
                    lhsT=wslice(wa, k, t),
                    rhs=xt[k][:, ts(sb, 512)],
                    start=(k == 0),
                    stop=(k == KC - 1),
                )
                if interleave and which == "q" and k < KC - 1:
                    emit_warm(3)
            # bias-add + bf16 cast on DVE (keeps ACT free for exp).  The
            # first slice's bias-adds are pinned to high priority so the
            # scheduler cannot queue a wv-gated V-projection DVE op ahead
            # of them (that inversion stalls the whole exp stream behind
            # the late-arriving V weights).
            with tc.high_priority() if interleave else nullcontext():
                nc.vector.tensor_scalar(
                    qt[t][:, ts(sb, 512)], pjq, bqa[:, t : t + 1], None,
                    mybir.AluOpType.add,
                )
                nc.vector.tensor_scalar(
                    kt[t][:, ts(sb, 512)], pjk, bka[:, t : t + 1], None,
                    mybir.AluOpType.add,
                )

        def emit_v(s):
            # V tile s (natural layout), bias added, ones-augmented per head
            pj = psum.tile([128, 512], F32, name="pj", tag="acc", bufs=2)
            for k in range(KC):
                nc.tensor.matmul(
                    pj,
                    lhsT=xt[k][:, ts(s, 128)],
                    rhs=wv[k],
                    start=(k == 0),
                    stop=(k == KC - 1),
                )
            vta = vt[s].rearrange("p (h c) -> p h c", c=HD + 1)
            nc.vector.memset(vta[:, :, HD : HD + 1], 1.0)
            nc.vector.tensor_add(
                vta[:, :, 0:HD],
                pj.rearrange("p (h c) -> p h c", c=HD),
                bvb.rearrange("p (h c) -> p h c", c=HD),
            )

        pending_op = []   # (s, n) out-projection tiles awaiting drain
        op_engs = [nc.sync, nc.gpsimd]
        op_state = {"n": 0}

        def drain_op(k=1):
            for _ in range(k):
                if not pending_op:
                    return
                s, n = pending_op.pop(0)
                op = psum.tile([128, 512], F32, name="op", tag="acc", bufs=2)
                for t in range(NDQ):
                    nc.tensor.matmul(
                        op,
                        lhsT=ct[t][:, ts(s, 128)],
                        rhs=wo[t][:, ts(n, 512)],
                        start=(t == 0),
                        stop=(t == NDQ - 1),
                    )
                og = work.tile([128, 512], F32, name="og", tag="og", bufs=3)
                # PSUM reads must go through DVE (gpsimd cannot access PSUM)
                nc.vector.tensor_copy(og, op)
                op_engs[op_state["n"] % 2].dma_start(
                    out=out[ts(s, 128), ts(n, 512)], in_=og
                )
                op_state["n"] += 1

        def emit_attn_pair(qb, p, host_slice=(), inloop_from=None, final=False,
                           split_v=False, prio_scores=False):
            """One attention pair-block: scores/exp/mask/PV over all kv
            blocks of (qb, pair p), then normalization into ct[p].

            host_slice: list of (t, sb) projection slices for upcoming
            blocks, interleaved into this block's early kv iterations (a
            few matmuls per iteration) so the exp stream never pauses for
            a standalone projection stretch between blocks.  Tiny blocks
            (nkb=4) host nothing - their boundaries are already DVE-dense.
            inloop_from: drain an out-proj tile every other kv iteration
            from this index on."""
            ctp_a = psum.tile([HD + 1, 512], F32, name="ctp_a", tag="ctp", bufs=2)
            ctp_b = psum.tile([HD + 1, 512], F32, name="ctp_b", tag="ctp", bufs=2)
            nkb = 4 * qb + 4
            deferred_pv = []
            hosted = None
            if host_slice:
                # flat MM list over all hosted slices, slice-sequential so
                # the acc PSUM ring only ever holds one open slice pair
                hosted = [
                    (which, k, hst, hsb)
                    for hst, hsb in host_slice
                    for which in ("q", "k")
                    for k in range(KC)
                ]
                # only iterations before this block's own V-JIT region can
                # host (the V projection shares the acc PSUM ring)
                nslots = min(
                    8 if len(host_slice) == 1 else 12,
                    4 * qb if p == 0 else nkb,
                    nkb,
                )
                per_slot = -(-len(hosted) // nslots)
                hcur = {}
                hi = 0
            for kb in range(nkb):
                w = kb * 128 - qb * 512   # diagonal window offset
                w0 = max(w, 0)            # first needed q column
                # both heads' score blocks in one 2-bank PSUM tile,
                # narrowed to the causally needed columns [w0, 512)
                st = psum.tile([128, 1024], F32, name="st", tag="st", bufs=2)
                with tc.high_priority() if prio_scores else nullcontext():
                    nc.tensor.matmul(
                        st[:, w0:512],
                        lhsT=kt[p][0:64, ts(kb, 128)],
                        rhs=qt[p][0:64, qb * 512 + w0 : qb * 512 + 512],
                        start=True,
                        stop=True,
                    )
                    nc.tensor.matmul(
                        st[:, 512 + w0 : 1024],
                        lhsT=kt[p][64:128, ts(kb, 128)],
                        rhs=qt[p][64:128, qb * 512 + w0 : qb * 512 + 512],
                        start=True,
                        stop=True,
                    )
                # V tile / hosted projection / out-proj filler AFTER the
                # scores so their dependencies never block the exp stream
                if p == 0 and kb >= 4 * qb and not split_v:
                    # just-in-time V tile for this q-block's diagonal
                    emit_v(kb)
                if hosted is not None and hi < len(hosted):
                    for _ in range(per_slot):
                        if hi >= len(hosted):
                            break
                        which, k, hst, hsb = hosted[hi]
                        hi += 1
                        key = (which, hst, hsb)
                        if key not in hcur:
                            hcur[key] = psum.tile(
                                [128, 512], F32, name="pj" + which,
                                tag="acc", bufs=2,
                            )
                        wa = wqa if which == "q" else wka
                        nc.tensor.matmul(
                            hcur[key],
                            lhsT=wslice(wa, k, hst),
                            rhs=xt[k][:, ts(hsb, 512)],
                            start=(k == 0),
                            stop=(k == KC - 1),
                        )
                        if which == "k" and k == KC - 1:
                            # slice complete: close it now so the acc ring
                            # frees for the next slice / V tiles / drains
                            nc.vector.tensor_scalar(
                                qt[hst][:, ts(hsb, 512)], hcur[("q", hst, hsb)],
                                bqa[:, hst : hst + 1], None, mybir.AluOpType.add,
                            )
                            nc.vector.tensor_scalar(
                                kt[hst][:, ts(hsb, 512)], hcur[("k", hst, hsb)],
                                bka[:, hst : hst + 1], None, mybir.AluOpType.add,
                            )
                if (inloop_from is not None and kb >= inloop_from
                        and (kb - inloop_from) % 2 == 0):
                    drain_op(1)
                pt = work.tile([128, 1024], BF16, name="pt", tag="pt", bufs=10)
                if w0 >= 384:
                    # narrow enough that two per-head activations beat one
                    nc.scalar.activation(
                        pt[:, w0:512], st[:, w0:512], EXP, scale=SM_SCALE
                    )
                    nc.scalar.activation(
                        pt[:, 512 + w0 : 1024], st[:, 512 + w0 : 1024],
                        EXP, scale=SM_SCALE,
                    )
                elif w0 > 0:
                    # single activation shifted to skip head a's dead prefix
                    # (exp'd garbage in head b's prefix is never read)
                    nc.scalar.activation(
                        pt[:, w0:1024], st[:, w0:1024], EXP, scale=SM_SCALE
                    )
                else:
                    nc.scalar.activation(pt, st, EXP, scale=SM_SCALE)
                if w >= 0:
                    # diagonal block: mask the 128-wide staircase wedge
                    msl = cmask[:, 384:512]
                    nc.vector.tensor_mul(
                        pt[:, w : w + 128], pt[:, w : w + 128], msl
                    )
                    nc.vector.tensor_mul(
                        pt[:, 512 + w : 512 + w + 128],
                        pt[:, 512 + w : 512 + w + 128],
                        msl,
                    )
                if split_v:
                    deferred_pv.append((kb, w0, pt))
                    continue
                for ctp, h, c0 in ((ctp_a, 2 * p, 0), (ctp_b, 2 * p + 1, 512)):
                    nc.tensor.matmul(
                        ctp[:, w0:512],
                        lhsT=vt[kb][:, h * (HD + 1) : (h + 1) * (HD + 1)],
                        rhs=pt[:, c0 + w0 : c0 + 512],
                        start=(kb == 0),
                        stop=(kb == nkb - 1),
                    )
            if split_v:
                # all scores/exps were emitted unblocked; now project V and
                # run the attention-weighted accumulations (this path is
                # used for the very first block, where the V weights arrive
                # on the tail of the input DMA)
                for kb in range(nkb):
                    emit_v(kb)
                for kb, w0, pt in deferred_pv:
                    for ctp, h, c0 in ((ctp_a, 2 * p, 0), (ctp_b, 2 * p + 1, 512)):
                        nc.tensor.matmul(
                            ctp[:, w0:512],
                            lhsT=vt[kb][:, h * (HD + 1) : (h + 1) * (HD + 1)],
                            rhs=pt[:, c0 + w0 : c0 + 512],
                            start=(kb == 0),
                            stop=(kb == nkb - 1),
                        )
            if final:
                # latency-optimized closing chain: no SBUF staging (nothing
                # reuses these PSUM banks), reciprocals straight from ctp,
                # broadcast + normalize in 128-column chunks so the closing
                # out-projection (which consumes ct chunk by chunk) starts
                # as early as possible.
                recs = []
                # modest priority boost: slide the tiny lrow/rec chain
                # ahead of the last in-loop drains' og-copies on the DVE
                # queue (they otherwise add ~1.4us to the closing chain)
                with tc.high_priority(offset=40):
                    for ctp in (ctp_a, ctp_b):
                        lrow = work.tile([1, 512], F32, name="lrow", tag="lrow", bufs=4)
                        nc.vector.tensor_copy(lrow, ctp[HD : HD + 1, :])
                        rec = work.tile([1, 512], F32, name="rec", tag="rec", bufs=4)
                        nc.vector.reciprocal_approx_fast(rec, lrow)
                        recs.append(rec)
                bcs = []
                for rec in recs:
                    bc = work.tile([HD, 512], F32, name="bc", tag="bc", bufs=4)
                    bcs.append(bc)
                for sc in range(4):
                    for bc, rec in zip(bcs, recs):
                        nc.gpsimd.partition_broadcast(
                            bc[:, ts(sc, 128)], rec[:, ts(sc, 128)]
                        )
                    for ctp, bc, h in (
                        (ctp_a, bcs[0], 2 * p), (ctp_b, bcs[1], 2 * p + 1)
                    ):
                        r0 = (h % 2) * HD
                        nc.vector.tensor_mul(
                            ct[p][r0 : r0 + HD, qb * 512 + sc * 128 : qb * 512 + (sc + 1) * 128],
                            ctp[0:HD, ts(sc, 128)],
                            bc[:, ts(sc, 128)],
                        )
                return
            # normalization. Order matters: both CT' copies go first so the
            # ctp PSUM banks are freed immediately (the next pair-block's PV
            # accumulation reuses them); the reciprocal chain (including the
            # ~1us gpsimd partition_broadcast) runs afterwards, off the
            # PSUM-reuse critical path.  l is re-read from the SBUF copy.
            ctn_ab = []
            for ctp, h in ((ctp_a, 2 * p), (ctp_b, 2 * p + 1)):
                ctn = work.tile([HD + 1, 512], F32, name="ctn", tag="ctn", bufs=6)
                nc.vector.tensor_copy(ctn, ctp)
                ctn_ab.append(ctn)
            recs = []
            for ctn in ctn_ab:
                # bounce l to partition 0: the custom-DVE reciprocal
                # mishandles base_partition != 0 on hardware
                lrow = work.tile([1, 512], F32, name="lrow", tag="lrow", bufs=4)
                nc.vector.tensor_copy(lrow, ctn[HD : HD + 1, :])
                rec = work.tile([1, 512], F32, name="rec", tag="rec", bufs=4)
                nc.vector.reciprocal_approx_fast(rec, lrow)
                recs.append(rec)
            bcs = []
            for rec in recs:
                bc = work.tile([HD, 512], F32, name="bc", tag="bc", bufs=4)
                nc.gpsimd.partition_broadcast(bc, rec)
                bcs.append(bc)
            # normalize muls on DVE (gpsimd would thrash its library state
            # switching between partition_broadcast and tensor ops); they
            # sit at the tail of the chain so the copies above already
            # freed the PSUM banks and the masks of the next block can
            # slot in between.
            for ctn, bc, h in ((ctn_ab[0], bcs[0], 2 * p), (ctn_ab[1], bcs[1], 2 * p + 1)):
                r0 = (h % 2) * HD
                nc.vector.tensor_mul(
                    ct[p][r0 : r0 + HD, ts(qb, 512)], ctn[0:HD, :], bc
                )

        # ---- phase 2: anti-diagonal schedule over (qb, pair) ----
        # Waves qb + p = const, deepest qb first.  Every projection slice
        # (t, sb) is emitted just before the attention block A(sb, t) — the
        # first block that needs it (A(qb,p) reads qt[p] block qb and kt[p]
        # blocks 0..qb).  This starts the exp stream as soon as x has
        # landed (~31us) and spreads the projection work in proportion to
        # wave depth (1.7us in wave 0 up to 6.8us in wave 3, back down to
        # 1.7us in wave 6), landing PE filler inside the increasingly
        # ACT-bound late waves.
        blocks = [
            (qb, wave - qb)
            for wave in range(NQT + NPAIR - 1)
            for qb in range(min(NQT - 1, wave), max(-1, wave - NPAIR), -1)
        ]
        # hosting map: block i pre-computes the projection slices of the
        # listed later blocks inside its kv loop.  Tiny nkb=4 blocks (i in
        # {0, 2, 5, 9}) host nothing; bigger earlier blocks take two.
        HOST = {1: [2, 3], 3: [4, 5], 4: [6], 6: [7, 8], 7: [9], 8: [10],
                10: [11, 12], 11: [13], 12: [14], 13: [15]}
        DRAIN_FROM = {10: 13, 11: 9, 13: 9, 14: 5, 15: 1}
        emit_proj_sb(0, 0, interleave=True)   # slice for the first block
        for i, (qb, p) in enumerate(blocks):
            wave = qb + p
            if i == 0:
                # A(0,0)'s kv loop is wall-to-wall V-JIT, so it can't host;
                # the next slice is emitted standalone right after it
                # split_v: all scores+exps before any V matmul, so the
                # exp stream isn't interleaved behind the wv-gated V
                # projections on the in-order PE queue
                emit_attn_pair(0, 0, prio_scores=True, split_v=True)
                emit_proj_sb(blocks[1][1], blocks[1][0])
            else:
                hs = [(blocks[j][1], blocks[j][0]) for j in HOST.get(i, [])]
                emit_attn_pair(
                    qb, p,
                    host_slice=hs,
                    inloop_from=DRAIN_FROM.get(i),
                    final=(i == len(blocks) - 1),
                )
            if wave >= 4:
                drain_op(1)
            if p == NPAIR - 1:
                # q-block qb complete: its out-projection becomes filler
                pending_op.extend(
                    (s, n) for s in range(4 * qb, 4 * qb + 4)
                    for n in range(D // 512)
                )

        # drain remaining out-projection tiles
        drain_op(len(pending_op) + 1)

    nc.compile()
    return nc


_CACHE = {}


def _get_nc():
    if "nc" not in _CACHE:
        _CACHE["nc"] = build_mha_nc()
    return _CACHE["nc"]


def make_in_maps(x, Wq, bq, Wk, bk, Wv, bv, Wo, bo):
    """Shard full inputs into the 8 per-core input maps."""
    bf16 = ml_dtypes.bfloat16
    x = np.asarray(x, dtype=np.float32)
    Wq = np.asarray(Wq, dtype=np.float32)
    Wk = np.asarray(Wk, dtype=np.float32)
    Wv = np.asarray(Wv, dtype=np.float32)
    Wo = np.asarray(Wo, dtype=np.float32)
    bq = np.asarray(bq, dtype=np.float32)
    bk = np.asarray(bk, dtype=np.float32)
    bv = np.asarray(bv, dtype=np.float32)

    in_maps = []
    for c in range(8):
        b, hg = divmod(c, 2)
        ch = slice(hg * 512, (hg + 1) * 512)
        in_maps.append(
            {
                "xT": np.ascontiguousarray(x[b].T).astype(bf16),
                "wqT": np.ascontiguousarray(Wq[ch, :].T).astype(bf16),
                "wkT": np.ascontiguousarray(Wk[ch, :].T).astype(bf16),
                "wvT": np.ascontiguousarray(Wv[ch, :].T).astype(bf16),
                "woT": np.ascontiguousarray(Wo[:, ch].T).astype(bf16),
                "bq": np.ascontiguousarray(bq[ch].reshape(512, 1)),
                "bk": np.ascontiguousarray(bk[ch].reshape(512, 1)),
                "bv": np.ascontiguousarray(bv[ch].reshape(1, 512)),
            }
        )
    return in_maps


def combine_outputs(results, bo):
    """Sum the two per-core partials for each batch and add bo."""
    bo = np.asarray(bo, dtype=np.float32)
    out = np.zeros((4, 2048, 1024), dtype=np.float32)
    for c in range(8):
        out[c // 2] += results[c]["out"]
    out += bo[None, None, :]
    return out


def kernel(x, Wq, bq, Wk, bk, Wv, bv, Wo, bo):
    nc = _get_nc()
    in_maps = make_in_maps(x, Wq, bq, Wk, bk, Wv, bv, Wo, bo)
    res = run_bass_kernel_spmd(nc, in_maps, core_ids=list(range(8)))
    return combine_outputs(res.results, bo)


# revision 60
# speedup vs baseline: 1.0028x; 1.0028x over previous
"""Trainium2 Bass kernel for causal multi-head attention (eval mode).

Problem shapes (hardcoded): x [B=4, S=2048, D=1024], 16 heads, head_dim 64,
weights Wq/Wk/Wv/Wo [1024, 1024], biases [1024].

reference:
  q/k/v = split_heads(x @ W.T + b)          -> [B, H, S, 64]
  scores = q k^T / 8, causal mask, softmax
  ctx = attn @ v, merge heads               -> [B, S, 1024]
  out = ctx @ Wo.T + bo

Sharding over 8 NeuronCores: core c handles batch b = c // 2 and head-group
hg = c % 2 (8 heads = 512 channels). Each core computes a partial output
[S, D] for its batch from its 8 heads; host sums the two partials per batch
and adds bo.

Per-core kernel (matmuls bf16, accumulation fp32 in PSUM):
  QT = Wq_s @ x_b^T  (+bq)   [512, S]   transposed layout, dq on partitions
  KT likewise
  V  = x_b @ Wv_s^T  (+bv)   [S, 512]   natural layout, each head's 64 cols
                                        augmented with a ones column (65)
  attention runs per head-PAIR (heads 2p, 2p+1 share a 128-partition tile):
    per kv block: ST [128 kv, 1024] holds both heads' score blocks
    (two row-group-packed matmuls, concurrent on the PE array).
    Causal narrowing: for diagonal kv blocks (window offset w >= 0) only
    q columns [w, 512) are computed/exp'd/accumulated; the 128-wide
    staircase wedge [w, w+128) is masked multiplicatively (DVE, bf16).
    P = exp(ST/8) via ACTIVATE -> bf16 (split into two per-head activations
    when w >= 256 so the ACT engine skips the unneeded prefix),
    CT'_h [65, 512] += [V_h | 1]^T P_h  (PSUM accumulate over kv blocks;
    row 64 = softmax denominator l),
    CT_h = CT'_h[0:64] * recip(l)
  out_partial = CT^T stack @ Wo_s^T  [S, D] fp32

Scheduling: the ACT engine's exp stream is the global co-bottleneck
(~167us of exp vs ~245us of PE work), so attention pair-blocks are
emitted in anti-diagonal waves over (qb, pair): (0,0), (1,0), (0,1),
(2,0), (1,1), (0,2), ...  A(qb,p) only needs qt[p] sequence block qb and
kt[p] blocks 0..qb, so the qt/kt projection is emitted in per-(t, sb)
512-column slices, each HOSTED inside an earlier attention block's kv
loop (a few matmuls per iteration, see the HOST map) — the exp stream
starts right after the first slice (~33us) and never pauses for a
standalone projection stretch.  Tiny nkb=4 blocks host nothing: their
boundaries are already dense with DVE work (masks + normalization), and
unloading them removed ~2.5us PE stalls at the following big-block
entries.  V tiles are emitted just-in-time inside the kv loop of each
q-block's first pair; out-projection tiles drain inside late kv loops
(DRAIN_FROM map) and one per pair boundary from wave 4 on.  Input DMAs
are balanced across the three DMA-capable queues (sync/scalar/gpsimd,
~100 GB/s each) in need order: wq/wk t=0 slices and the tiny biases
first on gpsimd (the first bias-add gates the exp stream), x split
3/3/2, wv right behind x (it gates the first PV accumulations), wo
last; warm-up matmuls (no data dependency) keep the PE busy and its
p-state ramped through the ~28us input window.

The pair-end normalization is ordered to free the ctp PSUM banks
immediately (both CT' copies first, then the reciprocal chain), keeping
the next pair's PV accumulation unblocked; the gpsimd partition_broadcast
(~1us) sits off the critical path, and gpsimd runs nothing else that
would force its library to reload.  The final block skips the SBUF
staging and normalizes straight out of PSUM in 128-column chunks so the
closing out-projection pipelines against it.

Softmax skips the row-max subtraction: scores/8 are O(+-10) for these
randn-scaled inputs, exp stays well inside fp32/bf16 range.
"""

from contextlib import ExitStack, nullcontext

import numpy as np
import ml_dtypes

import concourse.bacc as bacc
import concourse.bass as bass
import concourse.mybir as mybir
import concourse.tile as tile
from concourse.bass import ts
from concourse.bass_utils import run_bass_kernel_spmd

BF16 = mybir.dt.bfloat16
F32 = mybir.dt.float32
EXP = mybir.ActivationFunctionType.Exp
IDENT = mybir.ActivationFunctionType.Identity


def build_mha_nc(S=2048, D=1024, DQ=512, HD=64):
    """Build the per-core Bass program (identical on all 8 cores)."""
    H = DQ // HD          # heads per core (8)
    KC = D // 128         # contraction chunks over D (8)
    NDQ = DQ // 128       # dq tiles (4)
    NQT = S // 512        # q tiles, 512 wide (4)
    NS = S // 128         # s tiles (16)
    VW = H * (HD + 1)     # augmented V width (520)
    NPAIR = H // 2        # head pairs (4)
    SM_SCALE = 1.0 / np.sqrt(HD)

    nc = bacc.Bacc("TRN2", target_bir_lowering=False, debug=False)

    xT = nc.dram_tensor("xT", [D, S], BF16, kind="ExternalInput").ap()
    wqT = nc.dram_tensor("wqT", [D, DQ], BF16, kind="ExternalInput").ap()
    wkT = nc.dram_tensor("wkT", [D, DQ], BF16, kind="ExternalInput").ap()
    wvT = nc.dram_tensor("wvT", [D, DQ], BF16, kind="ExternalInput").ap()
    woT = nc.dram_tensor("woT", [DQ, D], BF16, kind="ExternalInput").ap()
    bq = nc.dram_tensor("bq", [DQ, 1], F32, kind="ExternalInput").ap()
    bk = nc.dram_tensor("bk", [DQ, 1], F32, kind="ExternalInput").ap()
    bv = nc.dram_tensor("bv", [1, DQ], F32, kind="ExternalInput").ap()
    out = nc.dram_tensor("out", [S, D], F32, kind="ExternalOutput").ap()

    with tile.TileContext(nc) as tc, ExitStack() as ctx:
        persist = ctx.enter_context(tc.tile_pool(name="persist", bufs=1))
        work = ctx.enter_context(tc.tile_pool(name="work", bufs=3))
        psum = ctx.enter_context(tc.tile_pool(name="psum", bufs=2, space="PSUM"))

        # ---- persistent inputs ----
        xt = [persist.tile([128, S], BF16, name=f"xt{k}", tag=f"xt{k}") for k in range(KC)]
        # wq/wk as single wide tiles so the DMA can load per-t column slices
        # (one 3D-AP DMA per t covering all 8 k-chunks)
        wqa = persist.tile([128, KC * DQ], BF16, name="wqa", tag="wqa")
        wka = persist.tile([128, KC * DQ], BF16, name="wka", tag="wka")
        wv = [persist.tile([128, DQ], BF16, name=f"wv{k}", tag=f"wv{k}") for k in range(KC)]
        wo = [persist.tile([128, D], BF16, name=f"wo{t}", tag=f"wo{t}") for t in range(NDQ)]
        bqa = persist.tile([128, NDQ], F32, name="bqa", tag="bqa")
        bka = persist.tile([128, NDQ], F32, name="bka", tag="bka")
        bvb = persist.tile([128, DQ], F32, name="bvb", tag="bvb")

        def wslice(wa, k, t):
            # [128, 128] lhsT slice of weight chunk k, output columns t
            return wa[:, k * DQ + t * 128 : k * DQ + (t + 1) * 128]

        def w_t_dma(eng, wa, wT, t):
            # one DMA loading the t-th 128-column slice of all 8 chunks
            dst = wa.rearrange("p (k c) -> p k c", c=DQ)[:, :, ts(t, 128)]
            src = bass.AP(
                tensor=wT.tensor,
                offset=t * 128,
                ap=[[DQ, 128], [DQ * 128, KC], [1, 128]],
            )
            eng.dma_start(out=dst, in_=src)

        # Input DMAs. Only sync (SP), scalar (ACT), and gpsimd can issue
        # DMAs; per-queue bandwidth is ~100 GB/s.  Balance by need time:
        # x (4 MB) split 3/3/2, the t=0 slices of wq/wk first on gpsimd so
        # the first projection can start as soon as x lands, later t slices
        # trickle in behind, wv/biases next, wo last.
        warm_in = persist.tile([128, 512], BF16, name="warm_in", tag="warm_in")
        nc.vector.memset(warm_in, 1.0)

        w_t_dma(nc.gpsimd, wqa, wqT, 0)
        w_t_dma(nc.gpsimd, wka, wkT, 0)
        # biases right behind the small t=0 weight slices on the gpsimd
        # queue (~17us): the very first bias-add gates the whole exp
        # stream, and behind the x megabytes it would land at ~37us
        nc.gpsimd.dma_start(
            out=bqa, in_=bass.AP(tensor=bq.tensor, offset=0, ap=[[1, 128], [128, NDQ]])
        )
        nc.gpsimd.dma_start(
            out=bka, in_=bass.AP(tensor=bk.tensor, offset=0, ap=[[1, 128], [128, NDQ]])
        )
        for k in range(3):
            nc.sync.dma_start(out=xt[k], in_=xT[ts(k, 128), :])
        for k in range(3, 6):
            nc.scalar.dma_start(out=xt[k], in_=xT[ts(k, 128), :])
        nc.gpsimd.dma_start(out=xt[6], in_=xT[ts(6, 128), :])
        nc.gpsimd.dma_start(out=xt[7], in_=xT[ts(7, 128), :])
        # bv: tiny single-row DMA + on-chip partition broadcast (a
        # 256 KB broadcast-DMA here would delay the PV-gating wv chunks
        # behind it on the sync queue by ~2.4us)
        bvr = persist.tile([1, DQ], F32, name="bvr", tag="bvr")
        nc.sync.dma_start(out=bvr, in_=bv[0:1, :])
        nc.gpsimd.partition_broadcast(bvb, bvr)
        # wv split 3/3/2 right behind x so the first V tiles can be
        # projected by ~33us (they gate the first block's PV stream)
        for k in range(3):
            nc.sync.dma_start(out=wv[k], in_=wvT[ts(k, 128), :])
        for k in range(3, 6):
            nc.scalar.dma_start(out=wv[k], in_=wvT[ts(k, 128), :])
        for k in range(6, KC):
            nc.gpsimd.dma_start(out=wv[k], in_=wvT[ts(k, 128), :])
        for t in range(1, NDQ):
            w_t_dma(nc.gpsimd, wqa, wqT, t)
            w_t_dma(nc.gpsimd, wka, wkT, t)
        for t in range(NDQ):
            nc.sync.dma_start(out=wo[t], in_=woT[ts(t, 128), :])

        # multiplicative causal staircase mask: M[i, c] = 1 if c >= i + 384
        # else 0. The wedge slice cmask[:, 384:512] has M[i, j] = (j >= i);
        # it is duplicated into both halves of a [128, 256] tile so one
        # 3D-AP multiply masks both heads' wedges in a single DVE op.
        cmask = persist.tile([128, 512], BF16, name="cmask", tag="cmask")
        nc.gpsimd.memset(cmask, 1.0)
        nc.gpsimd.affine_select(
            out=cmask,
            in_=cmask,
            compare_op=mybir.AluOpType.is_ge,
            fill=0.0,
            base=-384,
            pattern=[[1, 512]],
            channel_multiplier=-1,
        )
        cmaskd = persist.tile([128, 256], BF16, name="cmaskd", tag="cmaskd")
        nc.vector.tensor_copy(cmaskd[:, 0:128], cmask[:, 384:512])
        nc.vector.tensor_copy(cmaskd[:, 128:256], cmask[:, 384:512])

        # warm-up: dummy matmuls with no DMA dependency, covering the
        # input-DMA window and ramping the PE p-state. Results never read.
        def emit_warm(n):
            for _ in range(n):
                warm = psum.tile([128, 1024], F32, name="warm", tag="st", bufs=2)
                nc.tensor.matmul(
                    warm[:, 0:512],
                    lhsT=warm_in[:, 0:128],
                    rhs=warm_in,
                    start=True,
                    stop=True,
                )

        emit_warm(44)

        # ---- persistent intermediates ----
        qt = [persist.tile([128, S], BF16, name=f"qt{t}", tag=f"qt{t}") for t in range(NDQ)]
        kt = [persist.tile([128, S], BF16, name=f"kt{t}", tag=f"kt{t}") for t in range(NDQ)]
        vt = [persist.tile([128, VW], BF16, name=f"vt{s}", tag=f"vt{s}") for s in range(NS)]
        ct = [persist.tile([128, S], BF16, name=f"ct{t}", tag=f"ct{t}") for t in range(NDQ)]

        def emit_proj_sb(t, sb, interleave=False):
            # QT/KT projection for dq tile t, one 512-wide sequence block.
            # With interleave (the very first slice, racing the input DMA):
            # Q and K accumulations are opened together with the
            # xt[7]-dependent final matmuls pushed last, and warm filler is
            # woven between the early k-steps so the PE p-state stays ramped
            # while the x chunks trickle in.
            pjq = psum.tile([128, 512], F32, name="pjq", tag="acc", bufs=2)
            pjk = psum.tile([128, 512], F32, name="pjk", tag="acc", bufs=2)
            order = (
                [("q", k) for k in range(KC - 1)]
                + [("k", k) for k in range(KC - 1)]
                + [("q", KC - 1), ("k", KC - 1)]
                if interleave
                else [("q", k) for k in range(KC)] + [("k", k) for k in range(KC)]
            )
            for which, k in order:
                pj, wa = (pjq, wqa) if which == "q" else (pjk, wka)
                nc.tensor.matmul(
                    pj,
                    lhsT=wslice(wa, k, t),
                    rhs=xt[k][:, ts(sb, 512)],
                    start=(k == 0),
                    stop=(k == KC - 1),
                )
                if interleave and which == "q" and k < KC - 1:
                    emit_warm(3)
            # bias-add + bf16 cast on DVE (keeps ACT free for exp).  The
            # first slice's bias-adds are pinned to high priority so the
            # scheduler cannot queue a wv-gated V-projection DVE op ahead
            # of them (that inversion stalls the whole exp stream behind
            # the late-arriving V weights).
            with tc.high_priority() if interleave else nullcontext():
                nc.vector.tensor_scalar(
                    qt[t][:, ts(sb, 512)], pjq, bqa[:, t : t + 1], None,
                    mybir.AluOpType.add,
                )
                nc.vector.tensor_scalar(
                    kt[t][:, ts(sb, 512)], pjk, bka[:, t : t + 1], None,
                    mybir.AluOpType.add,
                )

        def emit_v(s):
            # V tile s (natural layout), bias added, ones-augmented per head
            pj = psum.tile([128, 512], F32, name="pj", tag="acc", bufs=2)
            for k in range(KC):
                nc.tensor.matmul(
                    pj,
                    lhsT=xt[k][:, ts(s, 128)],
                    rhs=wv[k],
                    start=(k == 0),
                    stop=(k == KC - 1),
                )
            vta = vt[s].rearrange("p (h c) -> p h c", c=HD + 1)
            nc.vector.memset(vta[:, :, HD : HD + 1], 1.0)
            nc.vector.tensor_add(
                vta[:, :, 0:HD],
                pj.rearrange("p (h c) -> p h c", c=HD),
                bvb.rearrange("p (h c) -> p h c", c=HD),
            )

        pending_op = []   # (s, n) out-projection tiles awaiting drain
        op_engs = [nc.sync, nc.gpsimd]
        op_state = {"n": 0}

        def drain_op(k=1):
            for _ in range(k):
                if not pending_op:
                    return
                s, n = pending_op.pop(0)
                op = psum.tile([128, 512], F32, name="op", tag="acc", bufs=2)
                for t in range(NDQ):
                    nc.tensor.matmul(
                        op,
                        lhsT=ct[t][:, ts(s, 128)],
                        rhs=wo[t][:, ts(n, 512)],
                        start=(t == 0),
                        stop=(t == NDQ - 1),
                    )
                og = work.tile([128, 512], F32, name="og", tag="og", bufs=3)
                # PSUM reads must go through DVE (gpsimd cannot access PSUM)
                nc.vector.tensor_copy(og, op)
                if op_state["n"] >= 30:
                    # the closing transfers gate the end-of-kernel barrier:
                    # split them across both DMA queues
                    nc.sync.dma_start(
                        out=out[ts(s, 128), n * 512 : n * 512 + 256],
                        in_=og[:, 0:256],
                    )
                    nc.gpsimd.dma_start(
                        out=out[ts(s, 128), n * 512 + 256 : (n + 1) * 512],
                        in_=og[:, 256:512],
                    )
                else:
                    op_engs[op_state["n"] % 2].dma_start(
                        out=out[ts(s, 128), ts(n, 512)], in_=og
                    )
                op_state["n"] += 1

        def emit_attn_pair(qb, p, host_slice=(), inloop_from=None, final=False,
                           split_v=False, prio_scores=False):
            """One attention pair-block: scores/exp/mask/PV over all kv
            blocks of (qb, pair p), then normalization into ct[p].

            host_slice: list of (t, sb) projection slices for upcoming
            blocks, interleaved into this block's early kv iterations (a
            few matmuls per iteration) so the exp stream never pauses for
            a standalone projection stretch between blocks.  Tiny blocks
            (nkb=4) host nothing - their boundaries are already DVE-dense.
            inloop_from: drain an out-proj tile every other kv iteration
            from this index on."""
            ctp_a = psum.tile([HD + 1, 512], F32, name="ctp_a", tag="ctp", bufs=2)
            ctp_b = psum.tile([HD + 1, 512], F32, name="ctp_b", tag="ctp", bufs=2)
            nkb = 4 * qb + 4
            deferred_pv = []
            hosted = None
            if host_slice:
                # flat MM list over all hosted slices, slice-sequential so
                # the acc PSUM ring only ever holds one open slice pair
                hosted = [
                    (which, k, hst, hsb)
                    for hst, hsb in host_slice
                    for which in ("q", "k")
                    for k in range(KC)
                ]
                # only iterations before this block's own V-JIT region can
                # host (the V projection shares the acc PSUM ring)
                nslots = min(
                    8 if len(host_slice) == 1 else 12,
                    4 * qb if p == 0 else nkb,
                    nkb,
                )
                per_slot = -(-len(hosted) // nslots)
                hcur = {}
                hi = 0
            for kb in range(nkb):
                w = kb * 128 - qb * 512   # diagonal window offset
                w0 = max(w, 0)            # first needed q column
                # both heads' score blocks in one 2-bank PSUM tile,
                # narrowed to the causally needed columns [w0, 512)
                st = psum.tile([128, 1024], F32, name="st", tag="st", bufs=2)
                with tc.high_priority() if prio_scores else nullcontext():
                    nc.tensor.matmul(
                        st[:, w0:512],
                        lhsT=kt[p][0:64, ts(kb, 128)],
                        rhs=qt[p][0:64, qb * 512 + w0 : qb * 512 + 512],
                        start=True,
                        stop=True,
                    )
                    nc.tensor.matmul(
                        st[:, 512 + w0 : 1024],
                        lhsT=kt[p][64:128, ts(kb, 128)],
                        rhs=qt[p][64:128, qb * 512 + w0 : qb * 512 + 512],
                        start=True,
                        stop=True,
                    )
                # V tile / hosted projection / out-proj filler AFTER the
                # scores so their dependencies never block the exp stream
                if p == 0 and kb >= 4 * qb and not split_v:
                    # just-in-time V tile for this q-block's diagonal
                    emit_v(kb)
                if hosted is not None and hi < len(hosted):
                    for _ in range(per_slot):
                        if hi >= len(hosted):
                            break
                        which, k, hst, hsb = hosted[hi]
                        hi += 1
                        key = (which, hst, hsb)
                        if key not in hcur:
                            hcur[key] = psum.tile(
                                [128, 512], F32, name="pj" + which,
                                tag="acc", bufs=2,
                            )
                        wa = wqa if which == "q" else wka
                        nc.tensor.matmul(
                            hcur[key],
                            lhsT=wslice(wa, k, hst),
                            rhs=xt[k][:, ts(hsb, 512)],
                            start=(k == 0),
                            stop=(k == KC - 1),
                        )
                        if which == "k" and k == KC - 1:
                            # slice complete: close it now so the acc ring
                            # frees for the next slice / V tiles / drains
                            nc.vector.tensor_scalar(
                                qt[hst][:, ts(hsb, 512)], hcur[("q", hst, hsb)],
                                bqa[:, hst : hst + 1], None, mybir.AluOpType.add,
                            )
                            nc.vector.tensor_scalar(
                                kt[hst][:, ts(hsb, 512)], hcur[("k", hst, hsb)],
                                bka[:, hst : hst + 1], None, mybir.AluOpType.add,
                            )
                if (inloop_from is not None and kb >= inloop_from
                        and (kb - inloop_from) % 2 == 0):
                    drain_op(1)
                pt = work.tile([128, 1024], BF16, name="pt", tag="pt", bufs=10)
                if w0 >= 384:
                    # narrow enough that two per-head activations beat one
                    nc.scalar.activation(
                        pt[:, w0:512], st[:, w0:512], EXP, scale=SM_SCALE
                    )
                    nc.scalar.activation(
                        pt[:, 512 + w0 : 1024], st[:, 512 + w0 : 1024],
                        EXP, scale=SM_SCALE,
                    )
                elif w0 > 0:
                    # single activation shifted to skip head a's dead prefix
                    # (exp'd garbage in head b's prefix is never read)
                    nc.scalar.activation(
                        pt[:, w0:1024], st[:, w0:1024], EXP, scale=SM_SCALE
                    )
                else:
                    nc.scalar.activation(pt, st, EXP, scale=SM_SCALE)
                if w >= 0:
                    # diagonal block: mask the 128-wide staircase wedge
                    msl = cmask[:, 384:512]
                    nc.vector.tensor_mul(
                        pt[:, w : w + 128], pt[:, w : w + 128], msl
                    )
                    nc.vector.tensor_mul(
                        pt[:, 512 + w : 512 + w + 128],
                        pt[:, 512 + w : 512 + w + 128],
                        msl,
                    )
                if split_v:
                    deferred_pv.append((kb, w0, pt))
                    continue
                for ctp, h, c0 in ((ctp_a, 2 * p, 0), (ctp_b, 2 * p + 1, 512)):
                    nc.tensor.matmul(
                        ctp[:, w0:512],
                        lhsT=vt[kb][:, h * (HD + 1) : (h + 1) * (HD + 1)],
                        rhs=pt[:, c0 + w0 : c0 + 512],
                        start=(kb == 0),
                        stop=(kb == nkb - 1),
                    )
            if split_v:
                # all scores/exps were emitted unblocked; now project V and
                # run the attention-weighted accumulations (this path is
                # used for the very first block, where the V weights arrive
                # on the tail of the input DMA)
                for kb in range(nkb):
                    emit_v(kb)
                for kb, w0, pt in deferred_pv:
                    for ctp, h, c0 in ((ctp_a, 2 * p, 0), (ctp_b, 2 * p + 1, 512)):
                        nc.tensor.matmul(
                            ctp[:, w0:512],
                            lhsT=vt[kb][:, h * (HD + 1) : (h + 1) * (HD + 1)],
                            rhs=pt[:, c0 + w0 : c0 + 512],
                            start=(kb == 0),
                            stop=(kb == nkb - 1),
                        )
            if final:
                # latency-optimized closing chain: no SBUF staging (nothing
                # reuses these PSUM banks), reciprocals straight from ctp,
                # broadcast + normalize in 128-column chunks so the closing
                # out-projection (which consumes ct chunk by chunk) starts
                # as early as possible.
                recs = []
                # modest priority boost: slide the tiny lrow/rec chain
                # ahead of the last in-loop drains' og-copies on the DVE
                # queue (they otherwise add ~1.4us to the closing chain)
                with tc.high_priority(offset=40):
                    for ctp in (ctp_a, ctp_b):
                        lrow = work.tile([1, 512], F32, name="lrow", tag="lrow", bufs=4)
                        nc.vector.tensor_copy(lrow, ctp[HD : HD + 1, :])
                        rec = work.tile([1, 512], F32, name="rec", tag="rec", bufs=4)
                        nc.vector.reciprocal_approx_fast(rec, lrow)
                        recs.append(rec)
                bcs = []
                for rec in recs:
                    bc = work.tile([HD, 512], F32, name="bc", tag="bc", bufs=4)
                    bcs.append(bc)
                for sc in range(4):
                    for bc, rec in zip(bcs, recs):
                        nc.gpsimd.partition_broadcast(
                            bc[:, ts(sc, 128)], rec[:, ts(sc, 128)]
                        )
                    for ctp, bc, h in (
                        (ctp_a, bcs[0], 2 * p), (ctp_b, bcs[1], 2 * p + 1)
                    ):
                        r0 = (h % 2) * HD
                        nc.vector.tensor_mul(
                            ct[p][r0 : r0 + HD, qb * 512 + sc * 128 : qb * 512 + (sc + 1) * 128],
                            ctp[0:HD, ts(sc, 128)],
                            bc[:, ts(sc, 128)],
                        )
                return
            # normalization. Order matters: both CT' copies go first so the
            # ctp PSUM banks are freed immediately (the next pair-block's PV
            # accumulation reuses them); the reciprocal chain (including the
            # ~1us gpsimd partition_broadcast) runs afterwards, off the
            # PSUM-reuse critical path.  l is re-read from the SBUF copy.
            ctn_ab = []
            for ctp, h in ((ctp_a, 2 * p), (ctp_b, 2 * p + 1)):
                ctn = work.tile([HD + 1, 512], F32, name="ctn", tag="ctn", bufs=6)
                nc.vector.tensor_copy(ctn, ctp)
                ctn_ab.append(ctn)
            recs = []
            for ctn in ctn_ab:
                # bounce l to partition 0: the custom-DVE reciprocal
                # mishandles base_partition != 0 on hardware
                lrow = work.tile([1, 512], F32, name="lrow", tag="lrow", bufs=4)
                nc.vector.tensor_copy(lrow, ctn[HD : HD + 1, :])
                rec = work.tile([1, 512], F32, name="rec", tag="rec", bufs=4)
                nc.vector.reciprocal_approx_fast(rec, lrow)
                recs.append(rec)
            bcs = []
            for rec in recs:
                bc = work.tile([HD, 512], F32, name="bc", tag="bc", bufs=4)
                nc.gpsimd.partition_broadcast(bc, rec)
                bcs.append(bc)
            # normalize muls on DVE (gpsimd would thrash its library state
            # switching between partition_broadcast and tensor ops); they
            # sit at the tail of the chain so the copies above already
            # freed the PSUM banks and the masks of the next block can
            # slot in between.
            for ctn, bc, h in ((ctn_ab[0], bcs[0], 2 * p), (ctn_ab[1], bcs[1], 2 * p + 1)):
                r0 = (h % 2) * HD
                nc.vector.tensor_mul(
                    ct[p][r0 : r0 + HD, ts(qb, 512)], ctn[0:HD, :], bc
                )

        # ---- phase 2: anti-diagonal schedule over (qb, pair) ----
        # Waves qb + p = const, deepest qb first.  Every projection slice
        # (t, sb) is emitted just before the attention block A(sb, t) — the
        # first block that needs it (A(qb,p) reads qt[p] block qb and kt[p]
        # blocks 0..qb).  This starts the exp stream as soon as x has
        # landed (~31us) and spreads the projection work in proportion to
        # wave depth (1.7us in wave 0 up to 6.8us in wave 3, back down to
        # 1.7us in wave 6), landing PE filler inside the increasingly
        # ACT-bound late waves.
        blocks = [
            (qb, wave - qb)
            for wave in range(NQT + NPAIR - 1)
            for qb in range(min(NQT - 1, wave), max(-1, wave - NPAIR), -1)
        ]
        # hosting map: block i pre-computes the projection slices of the
        # listed later blocks inside its kv loop.  Tiny nkb=4 blocks (i in
        # {0, 2, 5, 9}) host nothing; bigger earlier blocks take two.
        HOST = {1: [2, 3], 3: [4, 5], 4: [6], 6: [7, 8], 7: [9], 8: [10],
                10: [11, 12], 11: [13], 12: [14], 13: [15]}
        DRAIN_FROM = {10: 13, 11: 9, 13: 9, 14: 5, 15: 1}
        emit_proj_sb(0, 0, interleave=True)   # slice for the first block
        for i, (qb, p) in enumerate(blocks):
            wave = qb + p
            if i == 0:
                # A(0,0)'s kv loop is wall-to-wall V-JIT, so it can't host;
                # the next slice is emitted standalone right after it
                # split_v: all scores+exps before any V matmul, so the
                # exp stream isn't interleaved behind the wv-gated V
                # projections on the in-order PE queue
                emit_attn_pair(0, 0, prio_scores=True, split_v=True)
                emit_proj_sb(blocks[1][1], blocks[1][0])
            else:
                hs = [(blocks[j][1], blocks[j][0]) for j in HOST.get(i, [])]
                emit_attn_pair(
                    qb, p,
                    host_slice=hs,
                    inloop_from=DRAIN_FROM.get(i),
                    final=(i == len(blocks) - 1),
                )
            if wave >= 4:
                drain_op(1)
            if p == NPAIR - 1:
                # q-block qb complete: its out-projection becomes filler
                pending_op.extend(
                    (s, n) for s in range(4 * qb, 4 * qb + 4)
                    for n in range(D // 512)
                )

        # drain remaining out-projection tiles
        drain_op(len(pending_op) + 1)

    nc.compile()
    return nc


_CACHE = {}


def _get_nc():
    if "nc" not in _CACHE:
        _CACHE["nc"] = build_mha_nc()
    return _CACHE["nc"]


def make_in_maps(x, Wq, bq, Wk, bk, Wv, bv, Wo, bo):
    """Shard full inputs into the 8 per-core input maps."""
    bf16 = ml_dtypes.bfloat16
    x = np.asarray(x, dtype=np.float32)
    Wq = np.asarray(Wq, dtype=np.float32)
    Wk = np.asarray(Wk, dtype=np.float32)
    Wv = np.asarray(Wv, dtype=np.float32)
    Wo = np.asarray(Wo, dtype=np.float32)
    bq = np.asarray(bq, dtype=np.float32)
    bk = np.asarray(bk, dtype=np.float32)
    bv = np.asarray(bv, dtype=np.float32)

    in_maps = []
    for c in range(8):
        b, hg = divmod(c, 2)
        ch = slice(hg * 512, (hg + 1) * 512)
        in_maps.append(
            {
                "xT": np.ascontiguousarray(x[b].T).astype(bf16),
                "wqT": np.ascontiguousarray(Wq[ch, :].T).astype(bf16),
                "wkT": np.ascontiguousarray(Wk[ch, :].T).astype(bf16),
                "wvT": np.ascontiguousarray(Wv[ch, :].T).astype(bf16),
                "woT": np.ascontiguousarray(Wo[:, ch].T).astype(bf16),
                "bq": np.ascontiguousarray(bq[ch].reshape(512, 1)),
                "bk": np.ascontiguousarray(bk[ch].reshape(512, 1)),
                "bv": np.ascontiguousarray(bv[ch].reshape(1, 512)),
            }
        )
    return in_maps


def combine_outputs(results, bo):
    """Sum the two per-core partials for each batch and add bo."""
    bo = np.asarray(bo, dtype=np.float32)
    out = np.zeros((4, 2048, 1024), dtype=np.float32)
    for c in range(8):
        out[c // 2] += results[c]["out"]
    out += bo[None, None, :]
    return out


def kernel(x, Wq, bq, Wk, bk, Wv, bv, Wo, bo):
    nc = _get_nc()
    in_maps = make_in_maps(x, Wq, bq, Wk, bk, Wv, bv, Wo, bo)
    res = run_bass_kernel_spmd(nc, in_maps, core_ids=list(range(8)))
    return combine_outputs(res.results, bo)
